# revision 43
# baseline (speedup 1.0000x reference)
"""Trainium2 Bass kernel for nn_NodeModel (GNN message passing).

Math (see reference):
  mesh_agg = scatter_mean(mesh_edge_attr, mesh_dst, N)
  world_agg = scatter_mean(world_edge_attr, world_dst, N)
  h = relu(concat([x, mesh_agg, world_agg]) @ W1 + b1) @ W2 + b2
  out = x + LayerNorm(h) * gamma + beta

Strategy:
  - Host: nodes are globally sorted by (mesh_degree, world_degree) and packed
    into 784 windows of 128 nodes; windows are dealt to (core, slot) sorted by
    their max-degree profile so the 8 windows sharing one baked slot count are
    nearly identical.  Edges land in an ELL-ish layout of feature-major slot
    planes [feat=partition, node lane] in fp8-e4m3 with 1/deg pre-baked into
    the values, so the on-device scatter-sum directly yields the scatter-mean.
    One extra plane per slot carries the fp8-quantized per-node sum of fp8
    quantization residuals, recovering bf16-level accuracy at fp8 bandwidth.
  - Scatter-mean runs on the Tensor engine: plane pairs are summed with one
    fp8 DoubleRow matmul (lhsT=[plane pair], rhs=[I|I]) accumulated into PSUM
    (fp32), node-major; one contiguous accumulation group per PSUM tile.
  - MLP runs feature-major on the PE in bf16 (fp32 psum): rhs operands come
    from one batched xbar DMA tile-transpose of [mesh|world|x] bf16 blocks.
  - LayerNorm runs node-major after another DMA tile-transpose: bn_stats/
    bn_aggr on DVE, rsqrt via ACT sqrt + DVE reciprocal, normalize + residual
    on Pool; x input and output are bf16 in window-contiguous [P, wpc*D]
    layout so every DMA line is >=1KB contiguous.
  - Engine budget: SP=edge+x loads, ACT=transposes+ReLU+sqrt, DVE=world agg
    copies+bn stats, Pool=mesh agg copies+biases+normalize+residual+stores,
    PE=scatter+MLP.
  - All 8 cores run the same program on different data; host gathers,
    inverse-permutes, and upcasts the output.
"""

import os
import sys

import numpy as np

sys.path.insert(0, "/opt/trn_rl_repo")

import ml_dtypes

N_NODES = 100000
N_MESH = 600000
N_WORLD = 300000
D = 128
P = 128
C = 8  # cores
EPS = 1e-5
WPC = -(-N_NODES // (C * P))  # 98 windows per core
NW_TOT = C * WPC  # 784 global windows
NS = NW_TOT * P  # 100352 node slots
NB = int(os.environ.get("K_NB", "7"))  # windows per MLP batch

BF16 = ml_dtypes.bfloat16
FP8 = ml_dtypes.float8_e4m3

LAST_STATS = {}


# ----------------------------------------------------------------------------
# Host-side packing
# ----------------------------------------------------------------------------

def _tiles(a):
    return np.maximum(a, 1)


def _pack(x, mesh_edge_attr, world_edge_attr, mesh_dst, world_dst):
    """Build per-core device buffers + metadata."""
    mesh_dst = np.asarray(mesh_dst).astype(np.int64)
    world_dst = np.asarray(world_dst).astype(np.int64)

    dm = np.bincount(mesh_dst, minlength=N_NODES)
    dw = np.bincount(world_dst, minlength=N_NODES)

    # node order: sorted by (mesh degree, world degree)
    order = np.lexsort((dw, dm))
    pad = NS - N_NODES
    nw_tot = NW_TOT
    wpc = WPC
    ipos = np.empty(N_NODES, dtype=np.int64)
    ipos[order] = pad + np.arange(N_NODES)
    dms = np.zeros(NS, dtype=np.int64)
    dws = np.zeros(NS, dtype=np.int64)
    dms[pad:] = dm[order]
    dws[pad:] = dw[order]

    # per-window maxima, then deal windows to (core, slot) sorted by their
    # (Tm, Tw) profile so the 8 windows sharing a baked slot count are nearly
    # identical (fat dm-boundary windows cluster instead of poisoning slots).
    wmax_m = dms.reshape(nw_tot, P).max(axis=1)
    wmax_w = dws.reshape(nw_tot, P).max(axis=1)
    wrank = np.empty(nw_tot, dtype=np.int64)
    wrank[np.lexsort((wmax_w, wmax_m))] = np.arange(nw_tot)
    win_core = wrank % C          # [nw_tot]
    win_slot = wrank // C
    Tm = _tiles(np.zeros(wpc, np.int64))
    Tw = _tiles(np.zeros(wpc, np.int64))
    np.maximum.at(Tm, win_slot, _tiles(wmax_m))
    np.maximum.at(Tw, win_slot, _tiles(wmax_w))
    # +1 plane per slot: fp8 residual-correction plane (node's quantization
    # error sum goes at k = deg[node], always < Tm[slot]+1).
    Tm = Tm + 1
    Tw = Tw + 1
    # single interleaved buffer: window block = mesh planes then world planes
    coe = np.concatenate([[0], np.cumsum(P * (Tm + Tw))])  # len WPC+1
    com = coe[:-1]                  # mesh plane offset within buffer
    cow = coe[:-1] + P * Tm         # world plane offset
    CDT = int(coe[-1])

    buf = np.zeros(C * P * CDT, dtype=FP8)
    d_ar = np.arange(D, dtype=np.int64) * CDT

    def pack_edges(attr, dst, deg, co):
        # feature-major slot planes: buf[c, d, co[s] + k*P + n] = q(attr[e, d])
        # for edge e with dst node at (core c, prog-slot s, lane n), edge
        # slot k within that node; values pre-scaled by 1/deg so the device
        # scatter-sum yields the mean directly.
        M = dst.shape[0]
        perm = np.argsort(dst, kind="stable")
        starts = np.concatenate([[0], np.cumsum(deg)])
        dst_sorted = dst[perm]
        k = np.arange(M, dtype=np.int64) - starts[dst_sorted]
        i = ipos[dst_sorted]
        g = i // P
        n = i % P
        c = win_core[g]
        s = win_slot[g]
        base = c * (P * CDT) + co[s] + k * P + n

        rdeg = (1.0 / np.maximum(deg, 1)).astype(np.float32)
        scaled = np.ascontiguousarray(attr).astype(np.float32)[perm]
        scaled *= rdeg[dst_sorted][:, None]
        q8 = scaled.astype(FP8)
        CH = 120000
        for lo in range(0, M, CH):
            hi = min(lo + CH, M)
            idx = base[lo:hi, None] + d_ar[None, :]
            buf[idx] = q8[lo:hi]

        # per-node fp8 residual-correction plane at k = deg[node]
        resid = scaled - q8.astype(np.float32)
        uniq, first_idx = np.unique(dst_sorted, return_index=True)
        sums = np.add.reduceat(resid, first_idx, axis=0)
        rq = sums.astype(FP8)
        i2 = ipos[uniq]
        g2 = i2 // P
        n2 = i2 % P
        base_corr = (win_core[g2] * (P * CDT) + co[win_slot[g2]]
                     + deg[uniq] * P + n2)
        for lo in range(0, uniq.shape[0], CH):
            hi = min(lo + CH, uniq.shape[0])
            idx = base_corr[lo:hi, None] + d_ar[None, :]
            buf[idx] = rq[lo:hi]

    pack_edges(mesh_edge_attr, mesh_dst, dm, com)
    pack_edges(world_edge_attr, world_dst, dw, cow)
    edge_buf = buf.reshape(C, P, CDT)

    # permuted x per core, window-contiguous bf16: x_buf[c, p, s*D:(s+1)*D]
    i = ipos[order]
    g = i // P
    p = i % P
    c = win_core[g]
    s = win_slot[g]

    xb = np.ascontiguousarray(x, dtype=np.float32).astype(BF16)
    x_buf = np.zeros((C, P, wpc * D), dtype=BF16)
    x_buf[c[:, None], p[:, None],
          (s * D)[:, None] + np.arange(D)[None, :]] = xb[order]

    # feature-major copy for the MLP rhs: xT_buf[c, d, s*P + lane]
    xT_buf = np.zeros((C, P, wpc * P), dtype=BF16)
    xT_buf[c[:, None], np.arange(D)[None, :], (s * P + p)[:, None]] = xb[order]

    unperm = (c, p, s)  # out[order] = result[c, p, s*D:(s+1)*D]
    return dict(
        Tm=Tm, Tw=Tw, coe=coe, CDT=CDT, edge_buf=edge_buf,
        x_buf=x_buf, xT_buf=xT_buf, order=order, unperm=unperm, wpc=wpc,
    )


# ----------------------------------------------------------------------------
# Device program
# ----------------------------------------------------------------------------

def _build_program(Tm, Tw, coe, CDT, has_beta, has_gamma=True, wpc=WPC):
    from contextlib import ExitStack
    import concourse.bass as bass
    import concourse.tile as tile
    from concourse import bacc, mybir

    f32 = mybir.dt.float32
    bf16 = mybir.dt.bfloat16
    fp8 = mybir.dt.float8e4
    AF = mybir.ActivationFunctionType
    OP = mybir.AluOpType
    PM = mybir.MatmulPerfMode

    nc = bacc.Bacc("TRN2", target_bir_lowering=False, debug=False,
                   enable_asserts=False, num_devices=C)

    edge_d = nc.dram_tensor("edge_buf", [P, CDT], fp8, kind="ExternalInput").ap()
    x_d = nc.dram_tensor("x_buf", [P, wpc * D], bf16, kind="ExternalInput").ap()
    xT_d = nc.dram_tensor("xT_buf", [P, wpc * P], bf16, kind="ExternalInput").ap()
    # weights packed into one tensor: [w1a | w1b | w1c | w2] -- fewer PJRT
    # buffers per execute (dispatch overhead scales with operand count)
    cw_d = nc.dram_tensor("cw", [D, 4 * D], bf16, kind="ExternalInput").ap()
    cb_d = nc.dram_tensor("cb", [P, 2], f32, kind="ExternalInput").ap()
    if has_gamma:
        gb_d = nc.dram_tensor("gamma_bc", [P, NB * D], bf16,
                              kind="ExternalInput").ap()
    if has_beta:
        bb_d = nc.dram_tensor("beta_bc", [P, NB * D], f32, kind="ExternalInput").ap()
    out_d = nc.dram_tensor("out_buf", [P, wpc * D], bf16, kind="ExternalOutput").ap()

    with tile.TileContext(nc) as tc, ExitStack() as ctx:
        ctx.enter_context(nc.allow_low_precision(
            reason="fp8/bf16 intermediates are intentional; PSUM accumulates fp32"))
        const = ctx.enter_context(tc.tile_pool(name="const", bufs=1))
        epool = ctx.enter_context(tc.tile_pool(name="edges", bufs=6))
        xpool = ctx.enter_context(tc.tile_pool(name="xin", bufs=6))
        lpool = ctx.enter_context(tc.tile_pool(name="long", bufs=6))
        tpool = ctx.enter_context(tc.tile_pool(name="work", bufs=5))
        cpool = ctx.enter_context(tc.tile_pool(name="cwork", bufs=4))
        spool = ctx.enter_context(tc.tile_pool(name="stats", bufs=6))
        _r1 = int(os.environ.get("K_PSUM1", "3"))
        _r2 = int(os.environ.get("K_PSUM2", "1"))
        psumh1 = ctx.enter_context(tc.tile_pool(name="psumh1", bufs=_r1, space="PSUM"))
        psumh2 = ctx.enter_context(tc.tile_pool(name="psumh2", bufs=_r2, space="PSUM"))

        def cload(shape, dt, src, tag):
            t = const.tile(shape, dt, tag=tag)
            nc.sync.dma_start(t[:], src)
            return t

        cw = cload([D, 4 * D], bf16, cw_d, "cw")
        cb = cload([P, 2], f32, cb_d, "cb")
        w1a = cw[:, 0 * D:1 * D]
        w1b = cw[:, 1 * D:2 * D]
        w1c = cw[:, 2 * D:3 * D]
        w2 = cw[:, 3 * D:4 * D]
        b1 = cb[:, 0:1]
        b2 = cb[:, 1:2]
        gb = cload([P, NB * D], bf16, gb_d, "gb") if has_gamma else None
        if has_beta:
            bbt = cload([P, NB * D], f32, bb_d, "bbt")
        epsc = const.tile([P, 1], f32, tag="epsc")
        nc.gpsimd.memset(epsc[:], EPS)

        batches = []
        b0 = 0
        while b0 < wpc:
            batches.append((b0, min(NB, wpc - b0)))
            b0 += NB

        state = {}

        def stage_a(bi):
            """Loads + fused scatter-projection on PE: the W1 blocks project
            each fp8 edge plane straight into the h1 PSUM accumulation, so
            the aggregates never materialize."""
            s0, nb = batches[bi]
            col0, col1 = int(coe[s0]), int(coe[s0 + nb])

            eet = epool.tile([P, col1 - col0], fp8, tag="edges")
            # split the edge load across two DMA queues (SP + Pool): Pool's
            # other work is two batches old, so its queue rarely blocks the
            # prefetch, unlike ACT whose yn-transpose waits on the current
            # batch's MLP output
            mid = int(coe[s0 + (nb + 1) // 2])
            nc.sync.dma_start(eet[:, 0:mid - col0], edge_d[:, col0:mid])
            nc.gpsimd.dma_start(eet[:, mid - col0:col1 - col0],
                                edge_d[:, mid:col1])

            xt = xpool.tile([P, nb * D], bf16, tag="x")
            nc.sync.dma_start(xt[:], x_d[:, s0 * D:(s0 + nb) * D])
            xT = lpool.tile([P, nb * P], bf16, tag="xT")
            nc.sync.dma_start(xT[:], xT_d[:, s0 * P:(s0 + nb) * P])
            xTv = xT[:]

            cow = coe[:-1] + P * np.asarray(Tm)
            tot = (sum(int(Tm[s]) + int(Tw[s]) for s in range(s0, s0 + nb))
                   + 1)
            h1 = psumh1.tile([P, nb * D], f32, tag="h1")
            # ONE contiguous accumulation group per psum tile: start=True
            # clears accumulation state on hardware at bank granularity, so
            # only the first matmul into the tile may set it.
            # start=True zeroing is 2KB-bank-granular: the FIRST matmul
            # touching each bank of the h1 tile must set it
            banks_started = set()
            for wgt, T, off in ((w1b, Tm, coe[:-1]), (w1c, Tw, cow)):
                for j in range(nb):
                    s = s0 + j
                    t = int(T[s])
                    o = int(off[s]) - col0
                    bank = (j * D * 4) // 2048
                    for kk in range(t):
                        st_flag = bank not in banks_started
                        banks_started.add(bank)
                        nc.tensor.matmul(
                            h1[:, j * D:(j + 1) * D], wgt,
                            eet[:, o + kk * P:o + (kk + 1) * P],
                            start=st_flag, stop=False,
                            skip_group_check=True,
                        )
            ncol = nb * P
            for c0 in range(0, ncol, 512):
                c1 = min(c0 + 512, ncol)
                # psum matmul outputs cannot cross a 2KB bank boundary
                nc.tensor.matmul(h1[:, c0:c1], w1a, xTv[:, c0:c1],
                                 start=False, stop=(c1 == ncol),
                                 skip_group_check=True)
            state[bi] = dict(xt=xt, h1=h1, nb=nb)

        def stage_r(bi):
            """ReLU in its own stage, emitted a step before the h2 matmul so
            PE's h2 input is always ready in SBUF (no cross-engine stall)."""
            s0, nb = batches[bi]
            st = state[bi]
            h1 = st.pop("h1")
            h1s = tpool.tile([P, nb * D], bf16, tag="h1s")
            # relu(h1 + b1) on ACT (idle queue -> prompt, keeps PE fed)
            nc.scalar.activation(h1s[:], h1[:], AF.Relu, bias=b1)
            st["h1s"] = h1s

        def stage_b(bi):
            """Second MLP layer + transpose back to node-major."""
            s0, nb = batches[bi]
            st = state[bi]
            h1s = st.pop("h1s")
            h2 = psumh2.tile([P, nb * D], f32, tag="h2")
            ncol = nb * D
            for c0 in range(0, ncol, 512):
                c1 = min(c0 + 512, ncol)
                nc.tensor.matmul(h2[:, c0:c1], w2, h1s[:, c0:c1],
                                 start=True, stop=(c1 == ncol),
                                 skip_group_check=True)
            yT = tpool.tile([P, nb * D], bf16, tag="yT")
            nc.vector.tensor_scalar(yT[:], h2[:], b2, None, op0=OP.add)
            yn = tpool.tile([P, nb, D], bf16, tag="yn")
            # tail of ACT's step queue: nothing ready-at-step-start behind it
            nc.scalar.dma_start(yn[:], yT[:], transpose=True)
            st["yn"] = yn

        def stage_c(bi):
            """LayerNorm (node-major) + gamma/beta + residual + store."""
            s0, nb = batches[bi]
            st = state.pop(bi)
            yn, xt = st["yn"], st["xt"]

            mv = spool.tile([P, 2 * nb], f32, tag="mv")
            for j in range(nb):
                st6 = spool.tile([P, 6], f32, tag="st6")
                nc.vector.bn_stats(st6[:], yn[:, j, :])
                nc.vector.bn_aggr(mv[:, 2 * j:2 * j + 2], st6[:])
            # sd = sqrt(var + eps) ; a = 1/sd ; bb = -mu * a
            sd = spool.tile([P, nb], f32, tag="sd")
            nc.scalar.activation(sd[:], mv[:, 1::2], AF.Sqrt, bias=epsc[:, 0:1])
            av = spool.tile([P, nb], f32, tag="av")
            nc.vector.reciprocal(av[:], sd[:])
            bbv = spool.tile([P, nb], f32, tag="bbv")
            nc.vector.tensor_tensor(bbv[:], mv[:, 0::2], av[:], op=OP.mult)

            tn = cpool.tile([P, nb * D], bf16, tag="tn")
            for j in range(nb):
                # t = yn * a - mu * a  == (yn - mu) * rsqrt(var+eps)
                nc.gpsimd.tensor_scalar(
                    tn[:, j * D:(j + 1) * D], yn[:, j, :],
                    av[:, j:j + 1], bbv[:, j:j + 1],
                    op0=OP.mult, op1=OP.subtract,
                )
            if has_gamma:
                gn = cpool.tile([P, nb * D], bf16, tag="gn")
                nc.vector.tensor_tensor(gn[:], tn[:], gb[:, :nb * D], op=OP.mult)
            else:
                gn = tn
            on = cpool.tile([P, nb * D], bf16, tag="on")
            nc.gpsimd.tensor_tensor(on[:], gn[:], xt[:], op=OP.add)
            if has_beta:
                nc.gpsimd.tensor_tensor(on[:], on[:], bbt[:, :nb * D], op=OP.add)

            nc.gpsimd.dma_start(out_d[:, s0 * D:(s0 + nb) * D], on[:])

        # software-pipelined emission: A(b) | R(b-sr) | C(b-sc) | B(b-sb) --
        # relu a step ahead of its h2 matmul; stage_c emitted BEFORE stage_b
        # so no queue has an end-of-step-dependent op (yT, yn) ahead of work
        # that is ready at step start (bn chain, sqrt, edge prefetch)
        sr = int(os.environ.get("K_SKEWR", "1"))
        sb = int(os.environ.get("K_SKEWB", "2"))
        sc = int(os.environ.get("K_SKEWC", "4"))
        nbat = len(batches)
        for b in range(nbat + sc):
            if b < nbat:
                stage_a(b)
            if sr <= b < nbat + sr:
                stage_r(b - sr)
            if b >= sc:
                stage_c(b - sc)
            if sb <= b < nbat + sb:
                stage_b(b - sb)

    nc.compile()
    return nc


_PROGRAM_CACHE = {}


def _get_program(Tm, Tw, coe, CDT, has_beta, has_gamma, wpc=WPC):
    key = (tuple(Tm), tuple(Tw), bool(has_beta), bool(has_gamma), wpc)
    if key not in _PROGRAM_CACHE:
        _PROGRAM_CACHE[key] = _build_program(Tm, Tw, coe, CDT, has_beta,
                                             has_gamma, wpc)
    return _PROGRAM_CACHE[key]


# ----------------------------------------------------------------------------
# SPMD runner (PJRT over axon), with optional repeat timing
# ----------------------------------------------------------------------------

_RUNNER_CACHE = {}


def _make_runner(nc):
    import jax
    from jax.sharding import Mesh, PartitionSpec, NamedSharding
    from jax.experimental.shard_map import shard_map
    from concourse import mybir
    from concourse.bass2jax import (_bass_exec_p, install_neuronx_cc_hook,
                                    partition_id_tensor)

    install_neuronx_cc_hook()

    partition_name = (nc.partition_id_tensor.name
                      if nc.partition_id_tensor else None)
    in_names, out_names, out_avals = [], [], []
    for alloc in nc.m.functions[0].allocations:
        if not isinstance(alloc, mybir.MemoryLocationSet):
            continue
        name = alloc.memorylocations[0].name
        if alloc.kind == "ExternalInput":
            if name != partition_name:
                in_names.append(name)
        elif alloc.kind == "ExternalOutput":
            out_names.append(name)
            out_avals.append(jax.core.ShapedArray(
                tuple(alloc.tensor_shape), mybir.dt.np(alloc.dtype)))
    n_params = len(in_names)
    all_names = in_names + out_names
    if partition_name is not None:
        all_names = all_names + [partition_name]

    def _body(*args):
        operands = list(args)
        if partition_name is not None:
            operands.append(partition_id_tensor())
        outs = _bass_exec_p.bind(
            *operands,
            out_avals=tuple(out_avals),
            in_names=tuple(all_names),
            out_names=tuple(out_names),
            lowering_input_output_aliases=(),
            sim_require_finite=True,
            sim_require_nnan=True,
            nc=nc,
        )
        return tuple(outs)

    devices = jax.devices()[:C]
    mesh = Mesh(np.asarray(devices), ("core",))
    spec = PartitionSpec("core")
    n_out = len(out_names)
    fn = jax.jit(
        shard_map(_body, mesh=mesh,
                  in_specs=(spec,) * (n_params + n_out),
                  out_specs=(spec,) * n_out,
                  check_rep=False),
        keep_unused=True,
    )
    sharding = NamedSharding(mesh, spec)
    return fn, in_names, out_names, out_avals, sharding


def _run_spmd(nc, in_maps, time_iters=0):
    import jax
    import time

    key = id(nc)
    if key not in _RUNNER_CACHE:
        _RUNNER_CACHE[key] = _make_runner(nc)
    fn, in_names, out_names, out_avals, sharding = _RUNNER_CACHE[key]

    concat_in = [
        jax.device_put(
            np.concatenate([np.asarray(in_maps[c][n]) for c in range(C)], axis=0),
            sharding)
        for n in in_names
    ]
    concat_zero = [
        jax.device_put(np.zeros((C * a.shape[0], *a.shape[1:]), a.dtype), sharding)
        for a in out_avals
    ]
    args = concat_in + concat_zero
    out = fn(*args)
    jax.block_until_ready(out)

    if time_iters > 0:
        t0 = time.perf_counter()
        for _ in range(time_iters):
            out = fn(*args)
        jax.block_until_ready(out)
        t1 = time.perf_counter()
        LAST_STATS["wall_per_iter_ns"] = (t1 - t0) / time_iters * 1e9

    return [
        {n: np.asarray(out[i]).reshape(C, *out_avals[i].shape)[c]
         for i, n in enumerate(out_names)}
        for c in range(C)
    ]


# ----------------------------------------------------------------------------
# Entry point
# ----------------------------------------------------------------------------

def kernel(x, mesh_edge_attr, world_edge_attr, mesh_dst, world_dst,
           W1, b1, W2, b2, gamma, beta):
    x = np.asarray(x, dtype=np.float32)
    W1 = np.asarray(W1, dtype=np.float32)
    W2 = np.asarray(W2, dtype=np.float32)
    b1 = np.asarray(b1, dtype=np.float32)
    b2 = np.asarray(b2, dtype=np.float32)
    gamma = np.asarray(gamma, dtype=np.float32)
    beta = np.asarray(beta, dtype=np.float32)

    pk = _pack(x, np.asarray(mesh_edge_attr, dtype=np.float32),
               np.asarray(world_edge_attr, dtype=np.float32),
               mesh_dst, world_dst)

    has_beta = bool(np.any(beta != 0.0))
    has_gamma = not bool(np.all(gamma == 1.0))
    nc = _get_program(pk["Tm"], pk["Tw"], pk["coe"], pk["CDT"], has_beta,
                      has_gamma, wpc=pk["wpc"])

    cw = np.concatenate([W1[0:D], W1[D:2 * D], W1[2 * D:3 * D], W2],
                        axis=1).astype(BF16)
    cb = np.stack([b1, b2], axis=1).astype(np.float32)
    gamma_bc = np.broadcast_to(np.tile(gamma, NB).astype(BF16),
                               (P, NB * D)).copy()

    in_maps = []
    for c in range(C):
        m = {
            "edge_buf": pk["edge_buf"][c],
        }
        if has_gamma:
            m["gamma_bc"] = gamma_bc
        m.update({
            "x_buf": pk["x_buf"][c],
            "xT_buf": pk["xT_buf"][c],
            "cw": cw, "cb": cb,
        })
        if has_beta:
            m["beta_bc"] = np.broadcast_to(np.tile(beta, NB),
                                           (P, NB * D)).astype(np.float32).copy()
        in_maps.append(m)

    results = _run_spmd(nc, in_maps,
                        time_iters=int(os.environ.get("KERNEL_TIME_ITERS", "0")))

    # out_buf[c, p, s*D + d] -> node at (core c, slot s, lane p), feature d
    out_stack = np.stack([results[c]["out_buf"] for c in range(C)])
    out_stack = out_stack.reshape(C, P, pk["wpc"], D).astype(np.float32)
    c_idx, p_idx, s_idx = pk["unperm"]
    out = np.empty((N_NODES, D), dtype=np.float32)
    out[pk["order"]] = out_stack[c_idx, p_idx, s_idx]
    return out
